# revision 32
# baseline (speedup 1.0000x reference)
"""Trainium2 Bass kernel for nn_Attention (B=4, L=1024, D=768, H=12, DH=64).

FP8 DoubleRow version.  Math per (batch b, head n):
    A = q_n^T k_n                 [D, D]  (shared across batches)
    scores = x A x^T              [L, L]
    S = softmax(scores, -1);  out = S V^T  with V = v_n x^T

Precision scheme (validated in numpy sim):
    host:  q,k scaled by 1024 -> fp8;  x -> fp8 (and bf16 copy for the
           V path and row-sums);  v stays bf16.
    A_ps = q8^T k8 (f32 psum, = A_true*2^20, entries ~341 +- 11)
    A_f8 = fp8((A_ps - 341.333)*2)          <- mean-subtracted residual
    WT_f8 = fp8(A_f8^T x8 * 2^-5)           <- residual only
    sT_ps = x8 @ WT_f8  +  outer(r, 21.333*r)   (K=1 bf16 matmul adds the
           exact rank-1 mean term; r = rowsums of x from bf16)
    pt = exp(sT_ps * 2^-16) bf16
    R^T accumulated per l-block over m-blocks with vt_aug (V^T | ones),
    normalized by the ones-column sums.

Sharding: 48 (b,n) units over 8 cores as 4 batch-pairs x 4 head-triples;
heads 1,2 of each triple compute half of A per core and exchange halves
with a 2-rank AllGather that overlaps head-0 work (as in the bf16
baseline).
"""

from contextlib import ExitStack

import ml_dtypes
import numpy as np

import concourse.tile as tile
from concourse import bacc, mybir
from concourse.bass import ts, ds
from concourse.bass_utils import run_bass_kernel_spmd

# If BASS_TRACE is set in an environment that lacks antenv.axon_hooks,
# run_bass_kernel_spmd's trace path would fail on import; register a
# fallback holder (and re-create the NTFF hook boot() skips when the
# module is missing at sitecustomize time) so tracing works/degrades
# gracefully instead.
try:
    import antenv.axon_hooks  # noqa: F401
except Exception:  # pragma: no cover
    import sys
    import types

    import antenv

    _m = types.ModuleType("antenv.axon_hooks")
    _m._hook = None
    _m.set_axon_ntff_profile_hook = lambda h: setattr(_m, "_hook", h)
    _m.get_axon_ntff_profile_hook = lambda: _m._hook
    sys.modules["antenv.axon_hooks"] = _m
    antenv.axon_hooks = _m
    try:
        from trn_agent_boot.trn_boot import _ntff_profile_via_ctypes

        _hook = _ntff_profile_via_ctypes("/opt/axon/libaxon_pjrt.so")
        if _hook is not None:
            _m.set_axon_ntff_profile_hook(_hook)
    except Exception:
        pass

B, L, D, H = 4, 1024, 768, 12
DH = D // H          # 64
HPC = 3              # heads per core
BPC = 2              # batches per core
N_CORES = 8
DC = D // 128        # 6 chunks of the contraction/feature dim
LB = L // 128        # 8 l-blocks / m-blocks
DHA = DH + 1         # 65: head slice width in vt_aug (ones column at 64)
DHALF = D // 2       # 384: A-half width for the pairwise exchange
F32 = mybir.dt.float32
BF16 = mybir.dt.bfloat16
F8 = mybir.dt.float8e4
DR = mybir.MatmulPerfMode.DoubleRow
PAIR_GROUPS = [[0, 4], [1, 5], [2, 6], [3, 7]]

C_PS = 0.25 * (2.0 ** 20) / D      # 341.333: expected A_ps mean
SA2 = 2.0                          # A residual scale
SW = 2.0 ** -5                     # WT residual copy scale
CR_SCALE = C_PS * SA2 * SW         # 21.333: correction row scale
EXP_SCALE = 1.0 / ((2.0 ** 20) * SA2 * SW)   # 2^-16

_COMPILED = None


def _build():
    nc = bacc.Bacc(
        "TRN2",
        target_bir_lowering=False,
        debug=False,
        enable_asserts=False,
        num_devices=N_CORES,
    )
    xTb_ext = nc.dram_tensor("xTb", [BPC, D, L], BF16, kind="ExternalInput").ap()
    xT8_ext = nc.dram_tensor("xT8", [BPC, D, L], F8, kind="ExternalInput").ap()
    q0_ext = nc.dram_tensor("q0", [D, D], F8, kind="ExternalInput").ap()
    # heads 1,2: q columns restricted to this core's A-row-half; k full.
    qh_ext = nc.dram_tensor("qh", [2, D, DHALF], F8, kind="ExternalInput").ap()
    k0_ext = nc.dram_tensor("k0", [D, D], F8, kind="ExternalInput").ap()
    kf_ext = nc.dram_tensor("kf", [2, D, D], F8, kind="ExternalInput").ap()
    vT3_ext = nc.dram_tensor("vT3", [D, HPC * DH], BF16, kind="ExternalInput").ap()
    out_ext = nc.dram_tensor(
        "out_r", [BPC, L, HPC * DH], F32, kind="ExternalOutput"
    ).ap()

    with tile.TileContext(nc) as tc, ExitStack() as ctx:
        xtb_pool = ctx.enter_context(tc.tile_pool(name="xtb", bufs=1))
        xt8_pool = ctx.enter_context(tc.tile_pool(name="xt8", bufs=1))
        vt3_pool = ctx.enter_context(tc.tile_pool(name="vt3", bufs=1))
        vt_pool = ctx.enter_context(tc.tile_pool(name="vt", bufs=1))
        qk_pool = ctx.enter_context(tc.tile_pool(name="qk", bufs=1))
        a_pool = ctx.enter_context(tc.tile_pool(name="a", bufs=1))
        wt_pool = ctx.enter_context(tc.tile_pool(name="wt", bufs=2))
        pt_pool = ctx.enter_context(tc.tile_pool(name="pt", bufs=3))
        row_pool = ctx.enter_context(tc.tile_pool(name="row", bufs=1))
        corr_pool = ctx.enter_context(tc.tile_pool(name="corr", bufs=1))
        soft_pool = ctx.enter_context(tc.tile_pool(name="soft", bufs=2))
        out_pool = ctx.enter_context(tc.tile_pool(name="outp", bufs=1))
        warm_pool = ctx.enter_context(tc.tile_pool(name="warm", bufs=1))
        dram_pool = ctx.enter_context(tc.tile_pool(name="dram", bufs=1, space="DRAM"))
        ps_wt = ctx.enter_context(tc.tile_pool(name="ps_wt", bufs=3, space="PSUM"))
        ps_st = ctx.enter_context(tc.tile_pool(name="ps_st", bufs=3, space="PSUM"))
        ps_r = ctx.enter_context(tc.tile_pool(name="ps_r", bufs=2, space="PSUM"))

        # ---------- HAM warmup: dummy matmuls with minimal deps ----------
        # DVE memsets complete in ~0.2us, so the PE starts almost at t=0.
        wl = warm_pool.tile([128, 128], BF16, tag="wl")
        wr = warm_pool.tile([128, 512], BF16, tag="wr")
        nc.vector.memset(wl[:], 0.0)
        nc.vector.memset(wr[:], 0.0)
        wp = ps_wt.tile([128, 512], F32, tag="ps_wt")
        for _ in range(13):
            nc.tensor.matmul(wp[:], wl[:], wr[:], start=True, stop=True)

        # ---------- loads: one big 3D-AP DMA per tensor ----------
        def load3d(pool, tag, dram2d, width, dt):
            t = pool.tile([128, DC, width], dt, tag=tag)
            nc.sync.dma_start(
                t[:], dram2d.rearrange("(c p) w -> p c w", p=128)
            )
            return t

        # q0,k0 first (A(h0) is the first big PE work), then the A-halves
        # inputs, then the VT(b0)/rowsum deps, then the rest.
        q0_sb = load3d(qk_pool, "q0", q0_ext[:], D, F8)
        k0_sb = load3d(qk_pool, "k0", k0_ext[:], D, F8)
        qh_sb = [None, None]
        kf_sb = [None, None]
        for h in (1, 2):
            qh_sb[h - 1] = load3d(qk_pool, f"qh{h}", qh_ext[h - 1], DHALF, F8)
            kf_sb[h - 1] = load3d(qk_pool, f"kf{h}", kf_ext[h - 1], D, F8)
        xtb = [None, None]
        xt8 = [None, None]
        xtb[0] = load3d(xtb_pool, "xtb0", xTb_ext[0], L, BF16)
        vt3 = load3d(vt3_pool, "vt3", vT3_ext[:], HPC * DH, BF16)
        xt8[0] = load3d(xt8_pool, "xt80", xT8_ext[0], L, F8)
        xtb[1] = load3d(xtb_pool, "xtb1", xTb_ext[1], L, BF16)
        xt8[1] = load3d(xt8_pool, "xt81", xT8_ext[1], L, F8)

        # a_t[h]: [128, DC, D] fp8 residual A; partition = d within chunk j,
        # middle = chunk j, inner = d'.
        a_t = [
            a_pool.tile([128, DC, D], F8, tag=f"a{h}", name=f"a{h}")
            for h in range(HPC)
        ]

        # ---------- VT_aug projection + rowsum rows per batch ----------
        vt = [None, None]
        r_row = [None, None]
        cr_row = [None, None]
        ones_col = row_pool.tile([128, 1], BF16, tag="ones_col")
        nc.gpsimd.memset(ones_col[:], 1.0)

        def build_vt(bi):
            tiles = []
            for j in range(LB):
                p = ps_wt.tile([128, 512], F32, tag="ps_wt")
                for i in range(DC):
                    nc.tensor.matmul(
                        p[:, : HPC * DH],
                        xtb[bi][:, i, ts(j, 128)],
                        vt3[:, i, :],
                        start=(i == 0),
                        stop=(i == DC - 1),
                    )
                t = vt_pool.tile([128, HPC * DHA], BF16, tag=f"vt{bi}_{j}")
                nc.gpsimd.memset(t[:], 1.0)
                t3 = t[:].rearrange("p (h c) -> p h c", h=HPC)
                p3 = p[:, : HPC * DH].rearrange("p (h c) -> p h c", h=HPC)
                nc.vector.tensor_copy(t3[:, :, :DH], p3[:])
                tiles.append(t)
            vt[bi] = tiles

        def build_r(bi):
            # r_row = rowsums of x (bf16); cr_row = 21.333 * r_row
            rr = row_pool.tile([1, L], BF16, tag=f"r{bi}")
            cr = row_pool.tile([1, L], BF16, tag=f"cr{bi}")
            for n in range(2):
                p = ps_st.tile([128, 512], F32, tag="ps_st", name="prr")
                for j in range(DC):
                    nc.tensor.matmul(
                        p[:1, :],
                        ones_col[:],
                        xtb[bi][:, j, ts(n, 512)],
                        start=(j == 0),
                        stop=(j == DC - 1),
                    )
                nc.vector.tensor_copy(rr[:, ts(n, 512)], p[:1, :])
                nc.vector.tensor_scalar_mul(
                    cr[:, ts(n, 512)], p[:1, :], CR_SCALE
                )
            r_row[bi] = rr
            cr_row[bi] = cr

        # corr[bi]: precomputed rank-1 correction tiles, piece (mj, n) =
        # outer(r[mj-slice], cr[n-half]) as [128, 512] bf16.  Built with
        # K=1 matmul broadcasts + DVE copies; added into the scoresT psum
        # by one DVE op per half (cheaper than a K=1 matmul per half in
        # every unit: the correction only depends on the batch).
        corr = [None, None]

        def build_corr(bi):
            c_t = corr_pool.tile([128, LB, L], BF16, tag=f"c{bi}", name="c_t")
            for mj in range(LB):
                for n in range(2):
                    p = ps_st.tile([128, 512], F32, tag="ps_st", name="pcr")
                    nc.tensor.matmul(
                        p[:],
                        r_row[bi][:, ts(mj, 128)],
                        cr_row[bi][:, ts(n, 512)],
                        start=True,
                        stop=True,
                    )
                    nc.vector.tensor_copy(c_t[:, mj, ts(n, 512)], p[:])
            corr[bi] = c_t

        # ---- head 0: full residual A locally (fp8 DoubleRow) ----
        # jp outer / n inner: one DR weight load serves both d'-halves.
        # Scheduled first: q0/k0 are the first DMAs to land.
        for i in range(DC):
            pp = [None, None]
            for jp in range(DC // 2):
                for n in range(2):
                    if jp == 0:
                        pp[n] = ps_wt.tile(
                            [128, 512], F32, tag="ps_wt", name="pa0"
                        )
                    nc.tensor.matmul(
                        pp[n][:, :DHALF],
                        q0_sb[:, 2 * jp:2 * jp + 2, ts(i, 128)],
                        k0_sb[:, 2 * jp:2 * jp + 2, ts(n, DHALF)],
                        start=(jp == 0),
                        stop=(jp == DC // 2 - 1),
                        perf_mode=DR,
                    )
            for n in range(2):
                nc.vector.tensor_scalar(
                    a_t[0][:, i, ds(n * DHALF, DHALF)],
                    pp[n][:, :DHALF],
                    -C_PS,
                    SA2,
                    mybir.AluOpType.add,
                    mybir.AluOpType.mult,
                )

        # ---- heads 1, 2: this core's 384-row half of A (all 768 cols),
        # rows d in [384*bp, 384*bp+384) via q columns restricted to that
        # half (qh) against the full k (kf).  One DR weight load serves
        # the 512- and 256-wide column pieces.  Halves exchanged with a
        # 2-rank AllGather that overlaps head-0's WT/sT work.
        ah_dr = dram_pool.tile([2 * DHALF, D], F8, tag="ah_dr")
        gth_dr = dram_pool.tile([4 * DHALF, D], F8, tag="gth")
        for h in (1, 2):
            for io in range(HPC):
                pp = [None, None]
                for jp in range(DC // 2):
                    for n in range(2):
                        w = 512 if n == 0 else 256
                        if jp == 0:
                            pp[n] = ps_wt.tile(
                                [128, 512], F32, tag="ps_wt", name="pah"
                            )
                        nc.tensor.matmul(
                            pp[n][:, :w],
                            qh_sb[h - 1][:, 2 * jp:2 * jp + 2, ts(io, 128)],
                            kf_sb[h - 1][:, 2 * jp:2 * jp + 2, ds(512 * n, w)],
                            start=(jp == 0),
                            stop=(jp == DC // 2 - 1),
                            perf_mode=DR,
                        )
                ao = a_pool.tile([128, D], F8, tag=f"ao{h}_{io}")
                for n in range(2):
                    w = 512 if n == 0 else 256
                    nc.vector.tensor_scalar(
                        ao[:, ds(512 * n, w)],
                        pp[n][:, :w],
                        -C_PS,
                        SA2,
                        mybir.AluOpType.add,
                        mybir.AluOpType.mult,
                    )
                nc.sync.dma_start(
                    ah_dr[ds((h - 1) * DHALF + 128 * io, 128), :], ao[:]
                )
        nc.gpsimd.collective_compute(
            "AllGather",
            mybir.AluOpType.bypass,
            replica_groups=PAIR_GROUPS,
            ins=[ah_dr[:].opt()],
            outs=[gth_dr[:].opt()],
        )
        build_vt(0)
        build_r(0)
        build_corr(0)
        # gth rows: rank r's rows (abs chunks 3r..3r+2) at
        # 2*DHALF*r + (h-1)*DHALF + 128*io
        for h in (1, 2):
            for i in range(DC):
                rank, io = i // HPC, i % HPC
                nc.sync.dma_start(
                    a_t[h][:, i, :],
                    gth_dr[
                        ds(2 * DHALF * rank + (h - 1) * DHALF + 128 * io, 128), :
                    ],
                )

        # out accumulators: one [128, LB, 192] f32 tile per batch ->
        # a single output DMA per batch.
        out_sb = [
            out_pool.tile(
                [128, LB, HPC * DH], F32, tag=f"out{bi}", name=f"out{bi}"
            )
            for bi in range(BPC)
        ]

        for h in range(HPC):
            for bi in range(BPC):
                x8 = xt8[bi]
                # per-unit WT tile (double-buffered across units so the
                # next unit's WT copies do not WAR-stall on the previous
                # unit's trailing scoresT reads)
                wt_sb = wt_pool.tile([128, DC, L], F8, tag="wt", name="wt_sb")
                # ---- WT residual [d', l] (fp8 DR) ----
                # jp outer / n inner so each DoubleRow weight load serves
                # two matmuls (DR disables FWL, so LDWEIGHTS is the
                # bottleneck if reloaded per matmul).
                for i in range(DC):
                    pp = [None, None]
                    for jp in range(DC // 2):
                        for n in range(2):
                            if jp == 0:
                                pp[n] = ps_wt.tile(
                                    [128, 512], F32, tag="ps_wt", name="pwt"
                                )
                            nc.tensor.matmul(
                                pp[n][:],
                                a_t[h][:, 2 * jp:2 * jp + 2, ts(i, 128)],
                                x8[:, 2 * jp:2 * jp + 2, ts(n, 512)],
                                start=(jp == 0),
                                stop=(jp == DC // 2 - 1),
                                perf_mode=DR,
                            )
                    for n in range(2):
                        # scalar engine (activation copy): offloads the DVE,
                        # which carries the corr-adds + normalize.
                        nc.scalar.activation(
                            wt_sb[:, i, ts(n, 512)], pp[n][:],
                            mybir.ActivationFunctionType.Copy, scale=SW,
                        )
                    if h == 0 and bi == 1 and i == 0:
                        # xtb[1] has long arrived; fill the WT phase with
                        # the second batch's VT + rowsum + corr work.
                        build_r(1)
                        build_vt(1)
                        build_corr(1)

                # ---- scoresT half-blocks + exp + R accumulation (pipelined)
                # Each (mj, n) half is its own one-bank psum group: 3 DR
                # matmuls (weights = x8 slice, one LDW each) + the K=1
                # rank-1 correction matmul.  exp and the R matmuls for the
                # matching l-blocks chase one half behind the PE.
                # R accumulates across mj into 2 persistent psum banks; a
                # start=True zeroes the WHOLE 2KB bank (zero-region), so
                # only the first matmul of each bank sets it.
                rps = [
                    ps_r.tile([128, 4 * DHA], F32, tag="ps_r", name="rps")
                    for _ in range(2)
                ]
                pt_cur = [None]

                # For the LAST unit there is no following WT phase to absorb
                # the exp/DVE drain, so apply the rank-1 term with a K=1
                # matmul (the PE idles during that drain anyway) and keep
                # the DVE out of the tail's critical path.
                last_unit = (h == HPC - 1 and bi == BPC - 1)

                def scores_t_half(mj, n):
                    p = ps_st.tile([128, 512], F32, tag="ps_st", name="pst")
                    for jp in range(DC // 2):
                        nc.tensor.matmul(
                            p[:],
                            x8[:, 2 * jp:2 * jp + 2, ts(mj, 128)],
                            wt_sb[:, 2 * jp:2 * jp + 2, ts(n, 512)],
                            start=(jp == 0),
                            stop=(jp == DC // 2 - 1) and not last_unit,
                            perf_mode=DR,
                        )
                    if last_unit:
                        nc.tensor.matmul(
                            p[:],
                            r_row[bi][:, ts(mj, 128)],
                            cr_row[bi][:, ts(n, 512)],
                            start=False,
                            stop=True,
                        )
                    else:
                        # exact rank-1 mean term, precomputed per batch
                        # (gpsimd has no PSUM port, so this must be DVE)
                        nc.vector.tensor_tensor(
                            p[:], p[:], corr[bi][:, mj, ts(n, 512)],
                            mybir.AluOpType.add,
                        )
                    return p

                def emit_half(mj, n, p):
                    if n == 0:
                        pt_cur[0] = pt_pool.tile(
                            [128, L], BF16, tag="pt", name="pt"
                        )
                    t = pt_cur[0]
                    nc.scalar.activation(
                        t[:, ts(n, 512)], p[:],
                        mybir.ActivationFunctionType.Exp, scale=EXP_SCALE,
                    )
                    for j in range(4):
                        lb = 4 * n + j
                        nc.tensor.matmul(
                            rps[n][:, ds(DHA * j, DHA)],
                            t[:, ts(lb, 128)],
                            vt[bi][mj][:, ds(DHA * h, DHA)],
                            start=(mj == 0 and j == 0),
                            stop=(mj == LB - 1 and j == 3),
                        )

                halves = [(mj, n) for mj in range(LB) for n in range(2)]
                pending = None
                for mj, n in halves:
                    p = scores_t_half(mj, n)
                    if pending is not None:
                        emit_half(*pending)
                    pending = (mj, n, p)
                emit_half(*pending)

                # ---- normalize (bank-major) + (final head) output DMA ----
                for g in range(2):
                    for j in range(4):
                        lb = 4 * g + j
                        pr = rps[g]
                        recip = soft_pool.tile([128, 1], F32, tag="recip")
                        nc.vector.reciprocal(
                            recip[:], pr[:, ds(DHA * j + DH, 1)]
                        )
                        nc.vector.tensor_scalar_mul(
                            out_sb[bi][:, lb, ts(h, DH)],
                            pr[:, ds(DHA * j, DH)],
                            recip[:],
                        )
                    if h == HPC - 1:
                        nc.sync.dma_start(
                            out_ext[bi].rearrange("(c p) w -> p c w", p=128)[
                                :, ts(g, 4), :
                            ],
                            out_sb[bi][:, ts(g, 4), :],
                        )

    nc.compile()
    return nc


def kernel(x, k, q, v):
    global _COMPILED
    if _COMPILED is None:
        _COMPILED = _build()

    x = np.ascontiguousarray(x, dtype=np.float32)
    k = np.ascontiguousarray(k, dtype=np.float32)
    q = np.ascontiguousarray(q, dtype=np.float32)
    v = np.ascontiguousarray(v, dtype=np.float32)

    bf = ml_dtypes.bfloat16
    f8 = ml_dtypes.float8_e4m3
    xT = x.transpose(0, 2, 1)              # [B, D, L]
    xTb = xT.astype(bf)
    xT8 = xT.astype(f8)
    q8 = (q * 1024.0).astype(f8)
    k8 = (k * 1024.0).astype(f8)
    vb = v.transpose(2, 0, 1).astype(bf)   # [D, H, DH]
    in_maps = []
    for c in range(N_CORES):
        bp, t = c // 4, c % 4
        hs = slice(HPC * t, HPC * (t + 1))
        h0 = HPC * t
        cols = slice(DHALF * bp, DHALF * (bp + 1))
        in_maps.append(
            {
                "xTb": np.ascontiguousarray(xTb[BPC * bp: BPC * (bp + 1)]),
                "xT8": np.ascontiguousarray(xT8[BPC * bp: BPC * (bp + 1)]),
                "q0": np.ascontiguousarray(q8[h0]),
                "qh": np.ascontiguousarray(q8[h0 + 1: h0 + 3, :, cols]),
                "k0": np.ascontiguousarray(k8[h0]),
                "kf": np.ascontiguousarray(k8[h0 + 1: h0 + 3]),
                "vT3": np.ascontiguousarray(vb[:, hs].reshape(D, HPC * DH)),
            }
        )

    res = run_bass_kernel_spmd(_COMPILED, in_maps, core_ids=list(range(N_CORES)))

    out = np.empty((B, L, D), np.float32)
    for c in range(N_CORES):
        bp, t = c // 4, c % 4
        for bi in range(BPC):
            out[BPC * bp + bi, :, HPC * DH * t: HPC * DH * (t + 1)] = res.results[
                c
            ]["out_r"][bi]
    return out


if __name__ == "__main__":
    rng = np.random.default_rng(0)
    x = rng.standard_normal((B, L, D)).astype(np.float32)
    k = (rng.random((H, D, D)) / D).astype(np.float32)
    q = (rng.random((H, D, D)) / D).astype(np.float32)
    v = (rng.random((H, DH, D)) / D).astype(np.float32)
    o = kernel(x=x, k=k, q=q, v=v)
    print("out", o.shape, o.dtype)
